# revision 1
# baseline (speedup 1.0000x reference)
"""2D FDTD single-step kernel for Trainium2, SPMD across 8 NeuronCores.

Problem: Ez/Hx/Hy fields of shape (2, 1, 4096, 4096) fp32.
    Hx[:-1ry]  -= c_mu * (Ez[1:] - Ez[:-1])        (d/dx along NX)
    Hy[:-1col] += c_mu * (Ez[:,1:] - Ez[:,:-1])    (d/dy along NY)
    Ez[1:,1:]  += c_eps * ((HyN[i]-HyN[i-1]) - (HxN[j]-HxN[j-1]))

Strategy:
  * Flatten (B, 1, NX, NY) -> (8192, 4096) rows; shard 1024 rows/core, no halos.
  * Device processes independent [128, 1024]-col chunks. Cross-partition (NX)
    differences are computed on the TensorEngine as bidiagonal-matrix matmuls
    (exact for +-c coefficient pairs up to fp32 matmul rounding); free-dim (NY)
    differences are plain shifted access patterns on the VectorEngine.
  * Rows 0 and 127 of every 128-row tile and the first column of every
    1024-col chunk come out wrong on-device; they are recomputed on the host
    with numpy (96 rows + 3 cols per image - negligible).
"""

import numpy as np

import concourse.bacc as bacc
import concourse.tile as tile
from concourse import mybir
from concourse.bass_utils import run_bass_kernel_spmd

# ---- problem constants (hardcoded; must match the reference) ----
DT = 1e-11
DX = 0.01
EPS0 = 8.854e-12
MU0 = 4 * 3.1415926 * 1e-07
C_EPS = np.float32(DT / EPS0 / DX)
C_MU = np.float32(DT / MU0 / DX)

B, NX, NY = 2, 4096, 4096
N_CORES = 8
R = (B * NX) // N_CORES        # rows per core = 1024
TR = 128                        # tile rows (partition dim)
CW = 1024                       # column chunk width
N_RT = R // TR                  # row tiles per core = 8
N_CC = NY // CW                 # column chunks = 4
FP = mybir.dt.float32

_CACHE = {}


def _build_nc():
    nc = bacc.Bacc("TRN2", target_bir_lowering=False, debug=False)
    ez = nc.dram_tensor("ez", [R, NY], FP, kind="ExternalInput")
    hx = nc.dram_tensor("hx", [R, NY], FP, kind="ExternalInput")
    hy = nc.dram_tensor("hy", [R, NY], FP, kind="ExternalInput")
    wmu = nc.dram_tensor("wmu", [128, 128], FP, kind="ExternalInput")
    weps = nc.dram_tensor("weps", [128, 128], FP, kind="ExternalInput")
    ident = nc.dram_tensor("ident", [128, 128], FP, kind="ExternalInput")
    ezo = nc.dram_tensor("ezo", [R, NY], FP, kind="ExternalOutput")
    hxo = nc.dram_tensor("hxo", [R, NY], FP, kind="ExternalOutput")
    hyo = nc.dram_tensor("hyo", [R, NY], FP, kind="ExternalOutput")

    cm = float(C_MU)
    ce = float(C_EPS)
    MUL = mybir.AluOpType.mult
    ADD = mybir.AluOpType.add

    with tile.TileContext(nc) as tc:
        with (
            tc.tile_pool(name="const", bufs=1) as cpool,
            tc.tile_pool(name="io", bufs=3) as io,
            tc.tile_pool(name="tmp", bufs=2) as tmp,
            tc.tile_pool(name="ps", bufs=2, space="PSUM") as ps,
        ):
            twmu = cpool.tile([128, 128], FP, tag="twmu")
            tweps = cpool.tile([128, 128], FP, tag="tweps")
            tid = cpool.tile([128, 128], FP, tag="tid")
            nc.sync.dma_start(twmu[:], wmu[:])
            nc.sync.dma_start(tweps[:], weps[:])
            nc.sync.dma_start(tid[:], ident[:])

            for t in range(N_RT):
                r0 = t * TR
                for c in range(N_CC):
                    c0 = c * CW
                    last = c == N_CC - 1
                    # E tile carries one halo column: cols [c0, c0+CW] for
                    # interior chunks, [c0-1, c0+CW) for the last chunk.
                    eo = 1 if last else 0
                    te = io.tile([128, CW + 1], FP, tag="te")
                    th = io.tile([128, CW], FP, tag="th")
                    ty = io.tile([128, CW], FP, tag="ty")
                    nc.sync.dma_start(te[:], ez[r0:r0 + TR, c0 - eo:c0 - eo + CW + 1])
                    nc.sync.dma_start(th[:], hx[r0:r0 + TR, c0:c0 + CW])
                    nc.sync.dma_start(ty[:], hy[r0:r0 + TR, c0:c0 + CW])
                    e_main = te[:, eo:eo + CW]

                    # P1 = c_mu * (E[p+1]-E[p])  (row 127 garbage, host-fixed)
                    p1 = ps.tile([128, CW], FP, tag="p1")
                    for s in range(0, CW, 512):
                        nc.tensor.matmul(p1[:, s:s + 512], twmu[:],
                                         e_main[:, s:s + 512], start=True, stop=True)
                    # HxO = Hx - P1
                    tho = io.tile([128, CW], FP, tag="tho")
                    nc.vector.tensor_sub(tho[:], th[:], p1[:])
                    nc.sync.dma_start(hxo[r0:r0 + TR, c0:c0 + CW], tho[:])

                    # HyO[:, j] = Hy[:, j] + c_mu*(E[:, j+1]-E[:, j])
                    tyo = io.tile([128, CW], FP, tag="tyo")
                    t1 = tmp.tile([128, CW], FP, tag="t1")
                    wy = CW - 1 if last else CW  # last global col is unchanged
                    nc.vector.scalar_tensor_tensor(
                        t1[:, :wy], te[:, eo + 1:eo + 1 + wy], cm, ty[:, :wy], MUL, ADD)
                    nc.vector.scalar_tensor_tensor(
                        tyo[:, :wy], e_main[:, :wy], -cm, t1[:, :wy], MUL, ADD)
                    if last:
                        nc.scalar.copy(tyo[:, wy:CW], ty[:, wy:CW])
                    nc.sync.dma_start(hyo[r0:r0 + TR, c0:c0 + CW], tyo[:])

                    # P2 = c_eps*(HyO[p]-HyO[p-1]) + E   (row 0 garbage)
                    p2 = ps.tile([128, CW], FP, tag="p2")
                    for s in range(0, CW, 512):
                        nc.tensor.matmul(p2[:, s:s + 512], tweps[:],
                                         tyo[:, s:s + 512], start=True, stop=False)
                    for s in range(0, CW, 512):
                        nc.tensor.matmul(p2[:, s:s + 512], tid[:],
                                         e_main[:, s:s + 512], start=False, stop=True)

                    # EzO = P2 - c_eps*HxO[:, j] + c_eps*HxO[:, j-1]
                    t2 = tmp.tile([128, CW], FP, tag="t2")
                    nc.vector.scalar_tensor_tensor(
                        t2[:], tho[:], -ce, p2[:], MUL, ADD)
                    tezo = io.tile([128, CW], FP, tag="tezo")
                    nc.vector.scalar_tensor_tensor(
                        tezo[:, 1:CW], tho[:, 0:CW - 1], ce, t2[:, 1:CW], MUL, ADD)
                    # col 0 of the chunk: defined-but-wrong, host fixes
                    nc.scalar.copy(tezo[:, 0:1], t2[:, 0:1])
                    nc.sync.dma_start(ezo[r0:r0 + TR, c0:c0 + CW], tezo[:])
    nc.compile()
    return nc


def _get_nc():
    if "nc" not in _CACHE:
        _CACHE["nc"] = _build_nc()
    return _CACHE["nc"]


def _shift_mats():
    wmu = np.zeros((128, 128), np.float32)
    weps = np.zeros((128, 128), np.float32)
    for p in range(128):
        wmu[p, p] = -C_MU
        if p + 1 < 128:
            wmu[p + 1, p] = C_MU
        weps[p, p] = C_EPS
        if p >= 1:
            weps[p - 1, p] = -C_EPS
    ident = np.eye(128, dtype=np.float32)
    return wmu, weps, ident


def _host_fixups(Ez, Hx, Hy, EzO, HxO, HyO):
    """Recompute device-garbage boundary rows/cols with numpy (fp32)."""
    cm, ce = C_MU, C_EPS
    # Hx rows r = 127 mod 128 (incl. global last row, which is unchanged)
    rr = np.arange(127, NX, 128)
    ri = rr[:-1]
    HxO[:, :, ri, :] = Hx[:, :, ri, :] - cm * (Ez[:, :, ri + 1, :] - Ez[:, :, ri, :])
    HxO[:, :, NX - 1, :] = Hx[:, :, NX - 1, :]

    # Ez rows: r = 0 mod 128 (r>0) and r = 127 mod 128; recompute with the
    # corrected HxO and device HyO (all HyO rows are device-correct).
    rz = np.unique(np.concatenate([np.arange(0, NX, 128), np.arange(127, NX, 128)]))
    rz = rz[rz > 0]
    curl = (HyO[:, :, rz, 1:] - HyO[:, :, rz - 1, 1:]) - (
        HxO[:, :, rz, 1:] - HxO[:, :, rz, :-1])
    EzO[:, :, rz, 1:] = Ez[:, :, rz, 1:] + ce * curl
    # Ez interior chunk-boundary columns
    cz = np.arange(CW, NY, CW)
    curlc = (HyO[:, :, 1:, cz] - HyO[:, :, :-1, cz]) - (
        HxO[:, :, 1:, cz] - HxO[:, :, 1:, cz - 1])
    EzO[:, :, 1:, cz] = Ez[:, :, 1:, cz] + ce * curlc
    # unchanged global boundaries
    EzO[:, :, 0, :] = Ez[:, :, 0, :]
    EzO[:, :, :, 0] = Ez[:, :, :, 0]


def _run(Ez, Hx, Hy, trace=False):
    Ez = np.ascontiguousarray(np.asarray(Ez, dtype=np.float32))
    Hx = np.ascontiguousarray(np.asarray(Hx, dtype=np.float32))
    Hy = np.ascontiguousarray(np.asarray(Hy, dtype=np.float32))
    ez2 = Ez.reshape(B * NX, NY)
    hx2 = Hx.reshape(B * NX, NY)
    hy2 = Hy.reshape(B * NX, NY)
    wmu, weps, ident = _shift_mats()

    nc = _get_nc()
    in_maps = []
    for k in range(N_CORES):
        sl = slice(k * R, (k + 1) * R)
        in_maps.append({
            "ez": ez2[sl], "hx": hx2[sl], "hy": hy2[sl],
            "wmu": wmu, "weps": weps, "ident": ident,
        })
    res = run_bass_kernel_spmd(nc, in_maps, core_ids=list(range(N_CORES)),
                               trace=trace)
    _CACHE["last_results"] = res

    def gather(name):
        return np.concatenate(
            [res.results[k][name] for k in range(N_CORES)], axis=0
        ).reshape(B, 1, NX, NY)

    EzO = gather("ezo")
    HxO = gather("hxo")
    HyO = gather("hyo")
    _host_fixups(Ez, Hx, Hy, EzO, HxO, HyO)
    return EzO, HxO, HyO


def kernel(Ez, Hx, Hy):
    return _run(Ez, Hx, Hy, trace=False)


# revision 14
# speedup vs baseline: 1.2326x; 1.2326x over previous
"""2D FDTD single-step kernel for Trainium2, SPMD across 8 NeuronCores.

Problem: Ez/Hx/Hy fields of shape (2, 1, 4096, 4096) fp32.
    Hx[:-1ry]  -= c_mu * (Ez[1:] - Ez[:-1])        (d/dx along NX)
    Hy[:-1col] += c_mu * (Ez[:,1:] - Ez[:,:-1])    (d/dy along NY)
    Ez[1:,1:]  += c_eps * ((HyN[i]-HyN[i-1]) - (HxN[j]-HxN[j-1]))

Strategy:
  * Flatten (B, 1, NX, NY) -> (8192, 4096) rows; shard 1024 rows/core, no halos.
  * Device processes independent [128, 1024]-col chunks. Cross-partition (NX)
    differences are computed on the TensorEngine as bidiagonal-matrix matmuls
    (exact for +-c coefficient pairs up to fp32 matmul rounding); free-dim (NY)
    differences are plain shifted access patterns on the VectorEngine.
  * Rows 0 and 127 of every 128-row tile and the first column of every
    1024-col chunk come out wrong on-device; they are recomputed on the host
    with numpy (96 rows + 3 cols per image - negligible).
"""

import numpy as np

import concourse.bacc as bacc
import concourse.tile as tile
from concourse import mybir
from concourse.bass_utils import run_bass_kernel_spmd

# ---- problem constants (hardcoded; must match the reference) ----
DT = 1e-11
DX = 0.01
EPS0 = 8.854e-12
MU0 = 4 * 3.1415926 * 1e-07
C_EPS = np.float32(DT / EPS0 / DX)
C_MU = np.float32(DT / MU0 / DX)

B, NX, NY = 2, 4096, 4096
N_CORES = 8
R = (B * NX) // N_CORES        # rows per core = 1024
TR = 128                        # tile rows (partition dim)
CW = 2048                       # column chunk width
N_RT = R // TR                  # row tiles per core = 8
N_CC = NY // CW                 # column chunks = 4
FP = mybir.dt.float32

_CACHE = {}


def _build_nc():
    nc = bacc.Bacc("TRN2", target_bir_lowering=False, debug=False)
    ez = nc.dram_tensor("ez", [R, NY], FP, kind="ExternalInput")
    hx = nc.dram_tensor("hx", [R, NY], FP, kind="ExternalInput")
    hy = nc.dram_tensor("hy", [R, NY], FP, kind="ExternalInput")
    wmu = nc.dram_tensor("wmu", [128, 128], FP, kind="ExternalInput")
    weps = nc.dram_tensor("weps", [128, 128], FP, kind="ExternalInput")
    ezo = nc.dram_tensor("ezo", [R, NY], FP, kind="ExternalOutput")
    hxo = nc.dram_tensor("hxo", [R, NY], FP, kind="ExternalOutput")
    hyo = nc.dram_tensor("hyo", [R, NY], FP, kind="ExternalOutput")

    cm = float(C_MU)
    ce = float(C_EPS)
    MUL = mybir.AluOpType.mult
    ADD = mybir.AluOpType.add

    with tile.TileContext(nc) as tc:
        with (
            tc.tile_pool(name="const", bufs=1) as cpool,
            tc.tile_pool(name="io", bufs=3) as io,
            tc.tile_pool(name="tmp", bufs=2) as tmp,
            tc.tile_pool(name="ps", bufs=1, space="PSUM") as ps,
        ):
            twmu = cpool.tile([128, 128], FP, tag="twmu")
            tweps = cpool.tile([128, 128], FP, tag="tweps")
            nc.sync.dma_start(twmu[:], wmu[:])
            nc.sync.dma_start(tweps[:], weps[:])
            FPR = mybir.dt.float32r

            for t in range(N_RT):
                r0 = t * TR
                for c in range(N_CC):
                    c0 = c * CW
                    last = c == N_CC - 1
                    # E tile carries one halo column: cols [c0, c0+CW] for
                    # interior chunks, [c0-1, c0+CW) for the last chunk.
                    eo = 1 if last else 0
                    te = io.tile([128, CW + 1], FP, tag="te")
                    th = io.tile([128, CW], FP, tag="th")
                    ty = io.tile([128, CW], FP, tag="ty")
                    nc.sync.dma_start(te[:], ez[r0:r0 + TR, c0 - eo:c0 - eo + CW + 1])
                    nc.sync.dma_start(th[:], hx[r0:r0 + TR, c0:c0 + CW])
                    nc.sync.dma_start(ty[:], hy[r0:r0 + TR, c0:c0 + CW])
                    e_main = te[:, eo:eo + CW]

                    # P1 = c_mu * (E[p+1]-E[p])  (row 127 garbage, host-fixed)
                    p1 = ps.tile([128, CW], FP, tag="p1")
                    for s in range(0, CW, 512):
                        nc.tensor.matmul(p1[:, s:s + 512], twmu[:],
                                         e_main[:, s:s + 512],
                                         start=True, stop=True)
                    # HxO = Hx - P1
                    tho = io.tile([128, CW], FP, tag="tho")
                    nc.vector.tensor_sub(tho[:], th[:], p1[:])
                    nc.scalar.dma_start(hxo[r0:r0 + TR, c0:c0 + CW], tho[:])

                    # HyO[:, j] = Hy[:, j] + c_mu*(E[:, j+1]-E[:, j])
                    tyo = io.tile([128, CW], FP, tag="tyo")
                    t1 = tmp.tile([128, CW], FP, tag="t1")
                    wy = CW - 1 if last else CW  # last global col is unchanged
                    nc.vector.scalar_tensor_tensor(
                        t1[:, :wy], te[:, eo + 1:eo + 1 + wy], cm, ty[:, :wy], MUL, ADD)
                    nc.vector.scalar_tensor_tensor(
                        tyo[:, :wy], e_main[:, :wy], -cm, t1[:, :wy], MUL, ADD)
                    if last:
                        nc.vector.tensor_copy(tyo[:, wy:CW], ty[:, wy:CW])
                    nc.scalar.dma_start(hyo[r0:r0 + TR, c0:c0 + CW], tyo[:])

                    # P2 = c_eps*(HyO[p]-HyO[p-1])   (row 0 garbage)
                    p2 = ps.tile([128, CW], FP, tag="p2")
                    for s in range(0, CW, 512):
                        nc.tensor.matmul(p2[:, s:s + 512], tweps[:],
                                         tyo[:, s:s + 512], start=True, stop=True)

                    # EzO = E + P2 - c_eps*HxO[:, j] + c_eps*HxO[:, j-1]
                    t2 = tmp.tile([128, CW], FP, tag="t2")
                    nc.vector.scalar_tensor_tensor(
                        t2[:], tho[:], -ce, p2[:], MUL, ADD)
                    t3 = tmp.tile([128, CW], FP, tag="t3")
                    nc.vector.scalar_tensor_tensor(
                        t3[:, 1:CW], tho[:, 0:CW - 1], ce, t2[:, 1:CW], MUL, ADD)
                    tezo = io.tile([128, CW], FP, tag="tezo")
                    nc.vector.tensor_add(tezo[:, 1:CW], e_main[:, 1:CW], t3[:, 1:CW])
                    # col 0 of the chunk: defined-but-wrong, host fixes
                    nc.vector.tensor_copy(tezo[:, 0:1], t2[:, 0:1])
                    nc.scalar.dma_start(ezo[r0:r0 + TR, c0:c0 + CW], tezo[:])
    nc.compile()
    return nc


def _get_nc():
    if "nc" not in _CACHE:
        _CACHE["nc"] = _build_nc()
    return _CACHE["nc"]


def _shift_mats():
    wmu = np.zeros((128, 128), np.float32)
    weps = np.zeros((128, 128), np.float32)
    for p in range(128):
        wmu[p, p] = -C_MU
        if p + 1 < 128:
            wmu[p + 1, p] = C_MU
        weps[p, p] = C_EPS
        if p >= 1:
            weps[p - 1, p] = -C_EPS
    return wmu, weps


def _host_fixups(Ez, Hx, Hy, EzO, HxO, HyO):
    """Recompute device-garbage boundary rows/cols with numpy (fp32)."""
    cm, ce = C_MU, C_EPS
    # Hx rows r = 127 mod 128 (incl. global last row, which is unchanged)
    rr = np.arange(127, NX, 128)
    ri = rr[:-1]
    HxO[:, :, ri, :] = Hx[:, :, ri, :] - cm * (Ez[:, :, ri + 1, :] - Ez[:, :, ri, :])
    HxO[:, :, NX - 1, :] = Hx[:, :, NX - 1, :]

    # Ez rows: r = 0 mod 128 (r>0) and r = 127 mod 128; recompute with the
    # corrected HxO and device HyO (all HyO rows are device-correct).
    rz = np.unique(np.concatenate([np.arange(0, NX, 128), np.arange(127, NX, 128)]))
    rz = rz[rz > 0]
    curl = (HyO[:, :, rz, 1:] - HyO[:, :, rz - 1, 1:]) - (
        HxO[:, :, rz, 1:] - HxO[:, :, rz, :-1])
    EzO[:, :, rz, 1:] = Ez[:, :, rz, 1:] + ce * curl
    # Ez interior chunk-boundary columns
    cz = np.arange(CW, NY, CW)
    curlc = (HyO[:, :, 1:, cz] - HyO[:, :, :-1, cz]) - (
        HxO[:, :, 1:, cz] - HxO[:, :, 1:, cz - 1])
    EzO[:, :, 1:, cz] = Ez[:, :, 1:, cz] + ce * curlc
    # unchanged global boundaries
    EzO[:, :, 0, :] = Ez[:, :, 0, :]
    EzO[:, :, :, 0] = Ez[:, :, :, 0]


def _run(Ez, Hx, Hy, trace=False):
    Ez = np.ascontiguousarray(np.asarray(Ez, dtype=np.float32))
    Hx = np.ascontiguousarray(np.asarray(Hx, dtype=np.float32))
    Hy = np.ascontiguousarray(np.asarray(Hy, dtype=np.float32))
    ez2 = Ez.reshape(B * NX, NY)
    hx2 = Hx.reshape(B * NX, NY)
    hy2 = Hy.reshape(B * NX, NY)
    wmu, weps = _shift_mats()

    nc = _get_nc()
    in_maps = []
    for k in range(N_CORES):
        sl = slice(k * R, (k + 1) * R)
        in_maps.append({
            "ez": ez2[sl], "hx": hx2[sl], "hy": hy2[sl],
            "wmu": wmu, "weps": weps,
        })
    res = None
    last_err = None
    for attempt in range(4):
        try:
            res = run_bass_kernel_spmd(nc, in_maps, core_ids=list(range(N_CORES)),
                                       trace=trace)
            break
        except Exception as e:  # transient NRT device wedge recovers on retry
            last_err = e
            import time
            time.sleep(5)
            try:
                import jax
                jax.clear_caches()
                jax.extend.backend.clear_backends()
            except Exception:
                pass
    if res is None:
        raise last_err
    _CACHE["last_results"] = res

    def gather(name):
        return np.concatenate(
            [res.results[k][name] for k in range(N_CORES)], axis=0
        ).reshape(B, 1, NX, NY)

    EzO = gather("ezo")
    HxO = gather("hxo")
    HyO = gather("hyo")
    _host_fixups(Ez, Hx, Hy, EzO, HxO, HyO)
    return EzO, HxO, HyO


def kernel(Ez, Hx, Hy):
    return _run(Ez, Hx, Hy, trace=False)
